# revision 25
# baseline (speedup 1.0000x reference)
"""Trainium2 Bass kernel for nn_DiverseRegDCConv2d.

Per-sample dynamic 3x3 conv: filters are generated per sample from an
8-column weight bank (wgen[b] = se[b] @ bank.T), then applied as a
standard 256->256 conv on 28x28 with padding 1.

Sharding (8 cores): 4 batch-groups x 2 out-channel halves. Each core
handles 8 samples x 128 out channels; the weight bank half it needs is
replicated across the 4 batch-groups. No cross-device communication.

On-device filter generation trick: the bank half is pre-arranged on the
host into 128x128 stationary tiles whose partition axis is (n, g) with
n = bank column (8) and g = 16 different (k, o)-blocks; the streaming
operand is a block-diagonal arrangement of inputs_se (built on host,
64 KB). One matmul then produces filters for 16 (k,o)-pairs x 8 samples
with the conv's contraction axis (input channel) on PSUM partitions --
exactly the lhsT layout the conv matmuls need, so no on-device
transpose is ever required.

Precision: filter generation runs in fp16 (weights ~N(0, 0.02^2), well
inside fp16 range) with fp32 PSUM accumulation; the conv runs in
float32r (4-byte fp32 streamed at full PE rate). End-to-end relative
error vs the fp32 reference is ~3.7e-4.
"""

import sys

for _p in ("/opt/trn_rl_repo", "/root/.axon_site/_ro/trn_rl_repo"):
    if _p not in sys.path:
        sys.path.append(_p)

import numpy as np

import concourse.bass as bass
import concourse.mybir as mybir
from concourse import bacc
from concourse.bass_utils import run_bass_kernel_spmd
from concourse.tile import TileContext

B, C, O, KS, H, W, NUM = 32, 256, 256, 3, 28, 28, 8
P = 128
NCORES = 8
BG, OHALF = 4, 2          # batch-groups x out-channel halves
S = B // BG               # samples per core = 8
OC = O // OHALF           # out channels per core = 128
CC = C // P               # input-channel chunks = 2
G = 16                    # (k,o)-blocks per wgen matmul (with NUM=8 fills K=128)
NP = KS * KS * OC         # (k, o_local) pairs per c-chunk = 1152
NM = NP // G              # wgen matmuls per c-chunk = 72
F32 = mybir.dt.float32
F32R = mybir.dt.float32r
F16 = mybir.dt.float16

_NC = None


def _build_nc():
    nc = bacc.Bacc()
    x_d = nc.declare_dram_parameter("x", [S, C, H + 2, W + 2], F32R, isOutput=False)
    wp_d = nc.declare_dram_parameter("wp", [CC * NM, P, P], F16, isOutput=False)
    se_d = nc.declare_dram_parameter("sebd", [P, P], F16, isOutput=False)
    b_d = nc.declare_dram_parameter("bias", [P, 1], F32, isOutput=False)
    out_d = nc.declare_dram_parameter("out", [S, OC, H, W], F32, isOutput=True)

    with TileContext(nc) as tc:
        with (
            tc.tile_pool(name="constp", bufs=1) as constp,
            tc.tile_pool(name="wstream", bufs=3) as wstream,
            tc.tile_pool(name="slabp", bufs=1) as slabp,
            tc.tile_pool(name="xpool", bufs=1) as xpool,
            tc.tile_pool(name="outp", bufs=4) as outp,
            tc.tile_pool(name="accp", bufs=1) as accp,
            tc.tile_pool(name="wgps", bufs=2, space="PSUM") as wgps,
            tc.tile_pool(name="cvps", bufs=1, space="PSUM") as cvps,
        ):
            se_sb = constp.tile([P, P], F16)
            nc.sync.dma_start(out=se_sb, in_=se_d[:, :])
            bias_sb = constp.tile([P, 1], F32)
            nc.sync.dma_start(out=bias_sb, in_=b_d[:, :])

            # wgen slab: [c_part, cc, k, s, o] -- conv lhsT slices are
            # wg[:, cc, k, s, :], a contiguous [128, 128] tile.
            wg = slabp.tile([P, CC, KS * KS, S, P], F32R)

            # padded inputs: per (sample, c-chunk) a [128, 30, 30] tile.
            # Loads are interleaved into the wgen block loop so the weight
            # stream is not starved at kernel start.
            xpad = [[None] * CC for _ in range(S)]
            for s in range(S):
                for cc in range(CC):
                    xpad[s][cc] = xpool.tile(
                        [P, H + 2, W + 2], F32R, name=f"xpad_{s}_{cc}",
                        tag=f"xpad_{s}_{cc}",
                    )

            def emit_xload(s, cc):
                nc.sync.dma_start(
                    out=xpad[s][cc], in_=x_d[s, cc * P:(cc + 1) * P, :, :],
                )

            # xpad load schedule: a tile's DMA must be emitted before any
            # matmul that reads it (Tile deps follow program order). Keyed
            # by (cc, k) of the wgen block after which the loads go.
            xload_sched = {
                (0, 0): [(3, 0)], (0, 1): [(4, 0)], (0, 2): [(5, 0)],
                (0, 3): [(6, 0)], (0, 4): [(7, 0)],
                (0, 5): [(0, 1)], (0, 6): [(1, 1)], (0, 7): [(2, 1)],
                (1, 0): [(3, 1)], (1, 1): [(4, 1)], (1, 2): [(5, 1)],
                (1, 3): [(6, 1)], (1, 4): [(7, 1)],
            }

            HH = H // 2  # 14 output rows per matmul -> N = 392

            def emit_wgen(cc, k):
                # produce wg[:, cc, k, :, :] (8 o_hi blocks = 2 psum
                # groups); one DMA loads all 8 stationary tiles (fp16)
                t0 = cc * NM + k * 8
                wtb = wstream.tile([P, 8, P], F16)
                nc.sync.dma_start(
                    out=wtb,
                    in_=wp_d[t0:t0 + 8, :, :].rearrange("t p c -> p t c"),
                )
                for j in range(2):
                    m0 = k * 8 + j * 4
                    ps = wgps.tile([P, 4 * P], F32)
                    for i in range(4):
                        nc.tensor.matmul(
                            ps[:, i * P:(i + 1) * P], wtb[:, j * 4 + i, :],
                            se_sb, start=True, stop=True,
                        )
                    # psum free layout: (o_hi, s, g); slab wants (s, o_hi, g)
                    oh0 = m0 % 8
                    src = ps.rearrange("p (oh s g) -> p oh s g", oh=4, s=S, g=G)
                    dst = wg[:, cc, k, :, oh0 * G:(oh0 + 4) * G].rearrange(
                        "p s (oh g) -> p oh s g", g=G)
                    if j == 0:
                        nc.vector.tensor_copy(out=dst, in_=src)
                    else:
                        nc.scalar.activation(
                            dst, src, mybir.ActivationFunctionType.Copy,
                        )

            # Conv accumulation is split per c-chunk: each (cc, s, hi)
            # PSUM group is just 9 matmuls, with the cc=0 partial parked
            # in SBUF (acc) and combined during the cc=1 evacuation. Short
            # group lifetimes keep PSUM-bank pressure low so conv matmuls
            # can run inside the DMA-bound weight-streaming phase.
            acc = {
                (s, hi): accp.tile([P, HH, W], F32, name=f"acc_{s}_{hi}",
                                   tag=f"acc_{s}_{hi}")
                for s in range(S) for hi in range(2)
            }
            _tag = [0]

            def conv_psum():
                t = cvps.tile([P, HH, W], F32, name=f"cps_{_tag[0]}",
                              tag=f"cps_{_tag[0] % 6}")
                _tag[0] += 1
                return t

            def emit_conv_mm(cc, k, s, hi, pst):
                ky, kx = k // KS, k % KS
                h0 = hi * HH
                rhs = xpad[s][cc][:, h0 + ky:h0 + ky + HH, kx:kx + W]
                nc.tensor.matmul(
                    pst, wg[:, cc, k, s, :], rhs,
                    start=(k == 0), stop=(k == KS * KS - 1),
                    skip_group_check=True,
                )

            def emit_group_evac(cc, s, hi, pst):
                if cc == 0:
                    # park cc=0 partial (+bias) in SBUF, on the otherwise
                    # idle scalar engine
                    nc.scalar.activation(
                        acc[(s, hi)], pst,
                        mybir.ActivationFunctionType.Identity,
                        bias=bias_sb[:, 0:1],
                    )
                else:
                    ot = outp.tile([P, HH, W], F32, name=f"ot_{s}_{hi}",
                                   tag="ot")
                    nc.vector.tensor_tensor(
                        ot, pst, acc[(s, hi)], mybir.AluOpType.add,
                    )
                    nc.sync.dma_start(
                        out=out_d[s, :, hi * HH:hi * HH + HH, :], in_=ot,
                    )

            NPROG = 3  # samples whose groups accumulate progressively
            for s in range(NPROG):
                emit_xload(s, 0)
            for cc in range(CC):
                prog = {
                    (s, hi): conv_psum()
                    for s in range(NPROG) for hi in range(2)
                }
                for k in range(KS * KS):
                    emit_wgen(cc, k)
                    for sl in xload_sched.get((cc, k), ()):
                        emit_xload(*sl)
                    for s in range(NPROG):
                        for hi in range(2):
                            emit_conv_mm(cc, k, s, hi, prog[(s, hi)])
                for s in range(NPROG):
                    for hi in range(2):
                        emit_group_evac(cc, s, hi, prog[(s, hi)])
                # burst groups for the remaining samples
                for s in range(NPROG, S):
                    for hi in range(2):
                        pst = conv_psum()
                        for k in range(KS * KS):
                            emit_conv_mm(cc, k, s, hi, pst)
                        emit_group_evac(cc, s, hi, pst)

    nc.compile()
    return nc


def _get_nc():
    global _NC
    if _NC is None:
        _NC = _build_nc()
    return _NC


def _prep_core_inputs(inputs, inputs_se, weight, bias, bg, oh):
    # weight rows: r = o*(C*9) + c*9 + (ky*3+kx)  -> [O, C, 3, 3, NUM]
    wr = weight.reshape(O, C, KS, KS, NUM)
    wo = wr[oh * OC:(oh + 1) * OC]            # [128, 256, 3, 3, 8]
    p_arr = np.arange(NP)
    k_arr = p_arr // OC                       # k index per (m,g) pair
    o_arr = p_arr % OC
    t = wo[o_arr, :, k_arr // KS, k_arr % KS, :]     # [1152, 256, 8]
    wp = (
        t.reshape(NM, G, CC, P, NUM)
        .transpose(2, 0, 4, 1, 3)             # cc, m, n, g, c
        .reshape(CC * NM, P, P)
    )
    wp = np.ascontiguousarray(wp.astype(np.float16))

    se_core = inputs_se[bg * S:(bg + 1) * S]  # [8, 8] (s, n)
    sebd = np.zeros((NUM, G, S, G), dtype=np.float32)
    for g in range(G):
        sebd[:, g, :, g] = se_core.T
    sebd = sebd.reshape(P, P).astype(np.float16)

    x_core = np.pad(
        inputs[bg * S:(bg + 1) * S], ((0, 0), (0, 0), (1, 1), (1, 1))
    )
    return {
        "x": np.ascontiguousarray(x_core, dtype=np.float32),
        "wp": wp,
        "sebd": sebd,
        "bias": np.ascontiguousarray(
            bias[oh * OC:(oh + 1) * OC].reshape(OC, 1), dtype=np.float32
        ),
    }


def kernel(inputs, inputs_se, weight, bias):
    inputs = np.asarray(inputs, dtype=np.float32)
    inputs_se = np.asarray(inputs_se, dtype=np.float32)
    weight = np.asarray(weight, dtype=np.float32)
    bias = np.asarray(bias, dtype=np.float32)

    nc = _get_nc()
    in_maps = []
    for core in range(NCORES):
        bg, oh = core // OHALF, core % OHALF
        in_maps.append(_prep_core_inputs(inputs, inputs_se, weight, bias, bg, oh))

    res = run_bass_kernel_spmd(nc, in_maps, list(range(NCORES))).results

    out = np.empty((B, O, H, W), dtype=np.float32)
    for core in range(NCORES):
        bg, oh = core // OHALF, core % OHALF
        out[bg * S:(bg + 1) * S, oh * OC:(oh + 1) * OC] = res[core]["out"]
    return out


# revision 31
# speedup vs baseline: 1.0364x; 1.0364x over previous
"""Trainium2 Bass kernel for nn_DiverseRegDCConv2d.

Per-sample dynamic 3x3 conv: filters are generated per sample from an
8-column weight bank (wgen[b] = se[b] @ bank.T), then applied as a
standard 256->256 conv on 28x28 with padding 1.

Sharding (8 cores): 4 batch-groups x 2 out-channel halves. Each core
handles 8 samples x 128 out channels; the weight bank half it needs is
replicated across the 4 batch-groups. No cross-device communication.

On-device filter generation trick: the bank half is pre-arranged on the
host into 128x128 stationary tiles whose partition axis is (n, g) with
n = bank column (8) and g = 16 different (k, o)-blocks; the streaming
operand is a block-diagonal arrangement of inputs_se (built on host,
64 KB). One matmul then produces filters for 16 (k,o)-pairs x 8 samples
with the conv's contraction axis (input channel) on PSUM partitions --
exactly the lhsT layout the conv matmuls need, so no on-device
transpose is ever required.

Precision: filter generation and conv both run in fp16 operands
(weights ~N(0, 0.02^2) and x ~N(0,1), well inside fp16 range) with
fp32 PSUM accumulation throughout. End-to-end relative error vs the
fp32 reference is ~7e-4.
"""

import sys

for _p in ("/opt/trn_rl_repo", "/root/.axon_site/_ro/trn_rl_repo"):
    if _p not in sys.path:
        sys.path.append(_p)

import numpy as np

import concourse.bass as bass
import concourse.mybir as mybir
from concourse import bacc
from concourse.bass_utils import run_bass_kernel_spmd
from concourse.tile import TileContext

B, C, O, KS, H, W, NUM = 32, 256, 256, 3, 28, 28, 8
P = 128
NCORES = 8
BG, OHALF = 4, 2          # batch-groups x out-channel halves
S = B // BG               # samples per core = 8
OC = O // OHALF           # out channels per core = 128
CC = C // P               # input-channel chunks = 2
G = 16                    # (k,o)-blocks per wgen matmul (with NUM=8 fills K=128)
NP = KS * KS * OC         # (k, o_local) pairs per c-chunk = 1152
NM = NP // G              # wgen matmuls per c-chunk = 72
F32 = mybir.dt.float32
F32R = mybir.dt.float32r
F16 = mybir.dt.float16

_NC = None


def _build_nc():
    nc = bacc.Bacc()
    x_d = nc.declare_dram_parameter("x", [S, C, H + 2, W + 2], F16, isOutput=False)
    wp_d = nc.declare_dram_parameter("wp", [CC * NM, P, P], F16, isOutput=False)
    se_d = nc.declare_dram_parameter("sebd", [P, P], F16, isOutput=False)
    b_d = nc.declare_dram_parameter("bias", [P, 1], F32, isOutput=False)
    out_d = nc.declare_dram_parameter("out", [S, OC, H, W], F32, isOutput=True)

    with TileContext(nc) as tc:
        with (
            tc.tile_pool(name="constp", bufs=1) as constp,
            tc.tile_pool(name="wstream", bufs=12) as wstream,
            tc.tile_pool(name="slabp", bufs=1) as slabp,
            tc.tile_pool(name="xpool", bufs=1) as xpool,
            tc.tile_pool(name="outp", bufs=8) as outp,
            tc.tile_pool(name="accp", bufs=1) as accp,
            tc.tile_pool(name="wgps", bufs=2, space="PSUM") as wgps,
            tc.tile_pool(name="cvps", bufs=1, space="PSUM") as cvps,
        ):
            se_sb = constp.tile([P, P], F16)
            nc.sync.dma_start(out=se_sb, in_=se_d[:, :])
            bias_sb = constp.tile([P, 1], F32)
            nc.sync.dma_start(out=bias_sb, in_=b_d[:, :])

            # wgen slab: [c_part, cc, k, s, o] -- conv lhsT slices are
            # wg[:, cc, k, s, :], a contiguous [128, 128] tile.
            wg = slabp.tile([P, CC, KS * KS, S, P], F16)

            # padded inputs: per (sample, c-chunk) a [128, 30, 30] tile.
            # Loads are interleaved into the wgen block loop so the weight
            # stream is not starved at kernel start.
            xpad = [[None] * CC for _ in range(S)]
            for s in range(S):
                for cc in range(CC):
                    xpad[s][cc] = xpool.tile(
                        [P, H + 2, W + 2], F16, name=f"xpad_{s}_{cc}",
                        tag=f"xpad_{s}_{cc}",
                    )

            xdone = set()

            def emit_xload(s, cc):
                if (s, cc) in xdone:
                    return
                xdone.add((s, cc))
                nc.sync.dma_start(
                    out=xpad[s][cc], in_=x_d[s, cc * P:(cc + 1) * P, :, :],
                )

            # xpad load schedule: a tile's DMA must be emitted before any
            # matmul that reads it (Tile deps follow program order). Keyed
            # by (cc, k) of the wgen block after which the loads go.
            xload_sched = {
                (0, 0): [(3, 0)], (0, 1): [(4, 0)], (0, 2): [(5, 0)],
                (0, 3): [(6, 0)], (0, 4): [(7, 0)],
                (0, 5): [(0, 1)], (0, 6): [(1, 1)], (0, 7): [(2, 1)],
                (1, 0): [(3, 1)], (1, 1): [(4, 1)], (1, 2): [(5, 1)],
                (1, 3): [(6, 1)], (1, 4): [(7, 1)],
            }

            HH = H // 2  # 14 output rows per matmul -> N = 392

            def emit_wload(cc, k):
                # one DMA loads the block's 8 stationary tiles (fp16)
                t0 = cc * NM + k * 8
                wtb = wstream.tile([P, 8, P], F16, name=f"wtb_{cc}_{k}", tag="wtb")
                nc.sync.dma_start(
                    out=wtb,
                    in_=wp_d[t0:t0 + 8, :, :].rearrange("t p c -> p t c"),
                )
                return wtb

            def emit_wgen(cc, k, wtb):
                # produce wg[:, cc, k, :, :] (8 o_hi blocks = 2 psum groups)
                for j in range(2):
                    m0 = k * 8 + j * 4
                    ps = wgps.tile([P, 4 * P], F32)
                    for i in range(4):
                        nc.tensor.matmul(
                            ps[:, i * P:(i + 1) * P], wtb[:, j * 4 + i, :],
                            se_sb, start=True, stop=True,
                        )
                    # psum free layout: (o_hi, s, g); slab wants (s, o_hi, g)
                    oh0 = m0 % 8
                    src = ps.rearrange("p (oh s g) -> p oh s g", oh=4, s=S, g=G)
                    dst = wg[:, cc, k, :, oh0 * G:(oh0 + 4) * G].rearrange(
                        "p s (oh g) -> p oh s g", g=G)
                    if j == 0:
                        nc.vector.tensor_copy(out=dst, in_=src)
                    else:
                        nc.scalar.activation(
                            dst, src, mybir.ActivationFunctionType.Copy,
                        )

            # Conv accumulation is split per c-chunk: each (cc, s, hi)
            # PSUM group is just 9 matmuls, with the cc=0 partial parked
            # in SBUF (acc) and combined during the cc=1 evacuation. Short
            # group lifetimes keep PSUM-bank pressure low so conv matmuls
            # can run inside the DMA-bound weight-streaming phase.
            acc = {
                (s, hi): accp.tile([P, HH, W], F32, name=f"acc_{s}_{hi}",
                                   tag=f"acc_{s}_{hi}")
                for s in range(S) for hi in range(2)
            }
            _tag = [0]

            def conv_psum():
                t = cvps.tile([P, HH, W], F32, name=f"cps_{_tag[0]}",
                              tag=f"cps_{_tag[0] % 6}")
                _tag[0] += 1
                return t

            def emit_conv_mm(cc, k, s, hi, pst):
                ky, kx = k // KS, k % KS
                h0 = hi * HH
                rhs = xpad[s][cc][:, h0 + ky:h0 + ky + HH, kx:kx + W]
                nc.tensor.matmul(
                    pst, wg[:, cc, k, s, :], rhs,
                    start=(k == 0), stop=(k == KS * KS - 1),
                    skip_group_check=True,
                )

            def emit_group_evac(cc, s, hi, pst):
                if cc == 0:
                    # park cc=0 partial (+bias) in SBUF, on the otherwise
                    # idle scalar engine
                    nc.scalar.activation(
                        acc[(s, hi)], pst,
                        mybir.ActivationFunctionType.Identity,
                        bias=bias_sb[:, 0:1],
                    )
                else:
                    ot = outp.tile([P, HH, W], F32, name=f"ot_{s}_{hi}",
                                   tag="ot")
                    nc.vector.tensor_tensor(
                        ot, pst, acc[(s, hi)], mybir.AluOpType.add,
                    )
                    nc.sync.dma_start(
                        out=out_d[s, :, hi * HH:hi * HH + HH, :], in_=ot,
                    )

            NPROG = 3  # samples whose groups accumulate progressively
            for s in range(NPROG):
                emit_xload(s, 0)
            for cc in range(CC):
                prog = {
                    (s, hi): conv_psum()
                    for s in range(NPROG) for hi in range(2)
                }
                for k in range(KS * KS):
                    wtb = emit_wload(cc, k) if cc == 0 else wtb_next[k]
                    emit_wgen(cc, k, wtb)
                    for sl in xload_sched.get((cc, k), ()):
                        emit_xload(*sl)
                    for s in range(NPROG):
                        for hi in range(2):
                            emit_conv_mm(cc, k, s, hi, prog[(s, hi)])
                for s in range(NPROG):
                    for hi in range(2):
                        emit_group_evac(cc, s, hi, prog[(s, hi)])
                if cc == 0:
                    # prefetch ALL cc=1 weight blocks (and remaining x)
                    # before the cc=0 bursts, so the DMA engines stream
                    # the next phase while PE crunches the bursts.
                    wtb_next = [emit_wload(1, k) for k in range(KS * KS)]
                    for sl in [(s2, 1) for s2 in range(S)]:
                        if sl not in xdone:
                            emit_xload(*sl)
                # burst groups for the remaining samples
                for s in range(NPROG, S):
                    for hi in range(2):
                        pst = conv_psum()
                        for k in range(KS * KS):
                            emit_conv_mm(cc, k, s, hi, pst)
                        emit_group_evac(cc, s, hi, pst)

    nc.compile()
    return nc


def _get_nc():
    global _NC
    if _NC is None:
        _NC = _build_nc()
    return _NC


def _prep_core_inputs(inputs, inputs_se, weight, bias, bg, oh):
    # weight rows: r = o*(C*9) + c*9 + (ky*3+kx)  -> [O, C, 3, 3, NUM]
    wr = weight.reshape(O, C, KS, KS, NUM)
    wo = wr[oh * OC:(oh + 1) * OC]            # [128, 256, 3, 3, 8]
    p_arr = np.arange(NP)
    k_arr = p_arr // OC                       # k index per (m,g) pair
    o_arr = p_arr % OC
    t = wo[o_arr, :, k_arr // KS, k_arr % KS, :]     # [1152, 256, 8]
    wp = (
        t.reshape(NM, G, CC, P, NUM)
        .transpose(2, 0, 4, 1, 3)             # cc, m, n, g, c
        .reshape(CC * NM, P, P)
    )
    wp = np.ascontiguousarray(wp.astype(np.float16))

    se_core = inputs_se[bg * S:(bg + 1) * S]  # [8, 8] (s, n)
    sebd = np.zeros((NUM, G, S, G), dtype=np.float32)
    for g in range(G):
        sebd[:, g, :, g] = se_core.T
    sebd = sebd.reshape(P, P).astype(np.float16)

    x_core = np.pad(
        inputs[bg * S:(bg + 1) * S], ((0, 0), (0, 0), (1, 1), (1, 1))
    )
    return {
        "x": np.ascontiguousarray(x_core.astype(np.float16)),
        "wp": wp,
        "sebd": sebd,
        "bias": np.ascontiguousarray(
            bias[oh * OC:(oh + 1) * OC].reshape(OC, 1), dtype=np.float32
        ),
    }


def kernel(inputs, inputs_se, weight, bias):
    inputs = np.asarray(inputs, dtype=np.float32)
    inputs_se = np.asarray(inputs_se, dtype=np.float32)
    weight = np.asarray(weight, dtype=np.float32)
    bias = np.asarray(bias, dtype=np.float32)

    nc = _get_nc()
    in_maps = []
    for core in range(NCORES):
        bg, oh = core // OHALF, core % OHALF
        in_maps.append(_prep_core_inputs(inputs, inputs_se, weight, bias, bg, oh))

    res = run_bass_kernel_spmd(nc, in_maps, list(range(NCORES))).results

    out = np.empty((B, O, H, W), dtype=np.float32)
    for core in range(NCORES):
        bg, oh = core // OHALF, core % OHALF
        out[bg * S:(bg + 1) * S, oh * OC:(oh + 1) * OC] = res[core]["out"]
    return out


# revision 38
# speedup vs baseline: 1.2265x; 1.1834x over previous
"""Trainium2 Bass kernel for nn_DiverseRegDCConv2d.

Per-sample dynamic 3x3 conv: filters are generated per sample from an
8-column weight bank (wgen[b] = se[b] @ bank.T), then applied as a
standard 256->256 conv on 28x28 with padding 1.

Sharding (8 cores): 4 batch-groups x 2 out-channel halves. Each core
handles 8 samples x 128 out channels; the weight bank half it needs is
replicated across the 4 batch-groups. No cross-device communication.

On-device filter generation trick: the bank half is pre-arranged on the
host into 128x128 stationary tiles whose partition axis is (n, g) with
n = bank column (8) and g = 16 different (k, o)-blocks; the streaming
operand is a block-diagonal arrangement of inputs_se (built on host,
64 KB). One matmul then produces filters for 16 (k,o)-pairs x 8 samples
with the conv's contraction axis (input channel) on PSUM partitions --
exactly the lhsT layout the conv matmuls need, so no on-device
transpose is ever required.

Precision: filter generation and conv both run in fp16 operands
(weights ~N(0, 0.02^2) and x ~N(0,1), well inside fp16 range) with
fp32 PSUM accumulation throughout. End-to-end relative error vs the
fp32 reference is ~7e-4.
"""

import sys

for _p in ("/opt/trn_rl_repo", "/root/.axon_site/_ro/trn_rl_repo"):
    if _p not in sys.path:
        sys.path.append(_p)

import numpy as np

import concourse.bass as bass
import concourse.mybir as mybir
from concourse import bacc
from concourse.bass_utils import run_bass_kernel_spmd
from concourse.tile import TileContext

B, C, O, KS, H, W, NUM = 32, 256, 256, 3, 28, 28, 8
P = 128
NCORES = 8
BG, OHALF = 4, 2          # batch-groups x out-channel halves
S = B // BG               # samples per core = 8
OC = O // OHALF           # out channels per core = 128
CC = C // P               # input-channel chunks = 2
G = 16                    # (k,o)-blocks per wgen matmul (with NUM=8 fills K=128)
NP = KS * KS * OC         # (k, o_local) pairs per c-chunk = 1152
NM = NP // G              # wgen matmuls per c-chunk = 72
F32 = mybir.dt.float32
F32R = mybir.dt.float32r
F16 = mybir.dt.float16

_NC = None


def _build_nc():
    nc = bacc.Bacc()
    x_d = nc.declare_dram_parameter("x", [S, C, H + 2, W + 2], F16, isOutput=False)
    wp_d = nc.declare_dram_parameter("wp", [CC * NM, P, P], F16, isOutput=False)
    se_d = nc.declare_dram_parameter("sebd", [P, P], F16, isOutput=False)
    b_d = nc.declare_dram_parameter("bias", [P, 1], F32, isOutput=False)
    out_d = nc.declare_dram_parameter("out", [S, OC, H, W], F32, isOutput=True)

    with TileContext(nc) as tc:
        with (
            tc.tile_pool(name="constp", bufs=1) as constp,
            tc.tile_pool(name="wstream", bufs=18) as wstream,
            tc.tile_pool(name="slabp", bufs=1) as slabp,
            tc.tile_pool(name="xpool", bufs=1) as xpool,
            tc.tile_pool(name="outp", bufs=8) as outp,
            tc.tile_pool(name="accp", bufs=1) as accp,
            tc.tile_pool(name="wgps", bufs=2, space="PSUM") as wgps,
            tc.tile_pool(name="cvps", bufs=1, space="PSUM") as cvps,
        ):
            se_sb = constp.tile([P, P], F16)
            nc.sync.dma_start(out=se_sb, in_=se_d[:, :])
            bias_sb = constp.tile([P, 1], F32)
            nc.sync.dma_start(out=bias_sb, in_=b_d[:, :])

            # wgen slab: [c_part, cc, k, s, o] -- conv lhsT slices are
            # wg[:, cc, k, s, :], a contiguous [128, 128] tile.
            wg = slabp.tile([P, CC, KS * KS, S, P], F16)

            # padded inputs: per (sample, c-chunk) a [128, 30, 30] tile.
            # Loads are interleaved into the wgen block loop so the weight
            # stream is not starved at kernel start.
            xpad = [[None] * CC for _ in range(S)]
            for s in range(S):
                for cc in range(CC):
                    xpad[s][cc] = xpool.tile(
                        [P, H + 2, W + 2], F16, name=f"xpad_{s}_{cc}",
                        tag=f"xpad_{s}_{cc}",
                    )

            xdone = set()

            def emit_xload(s, cc):
                if (s, cc) in xdone:
                    return
                xdone.add((s, cc))
                nc.sync.dma_start(
                    out=xpad[s][cc], in_=x_d[s, cc * P:(cc + 1) * P, :, :],
                )

            HH = H // 2  # 14 output rows per matmul -> N = 392

            def emit_wload(cc, k):
                # one DMA loads the block's 8 stationary tiles (fp16)
                t0 = cc * NM + k * 8
                wtb = wstream.tile([P, 8, P], F16, name=f"wtb_{cc}_{k}", tag="wtb")
                nc.sync.dma_start(
                    out=wtb,
                    in_=wp_d[t0:t0 + 8, :, :].rearrange("t p c -> p t c"),
                )
                return wtb

            def emit_wgen(cc, k, wtb):
                # produce wg[:, cc, k, :, :] (8 o_hi blocks = 2 psum groups)
                for j in range(2):
                    m0 = k * 8 + j * 4
                    ps = wgps.tile([P, 4 * P], F32)
                    for i in range(4):
                        nc.tensor.matmul(
                            ps[:, i * P:(i + 1) * P], wtb[:, j * 4 + i, :],
                            se_sb, start=True, stop=True,
                        )
                    # psum free layout: (o_hi, s, g); slab wants (s, o_hi, g)
                    oh0 = m0 % 8
                    src = ps.rearrange("p (oh s g) -> p oh s g", oh=4, s=S, g=G)
                    dst = wg[:, cc, k, :, oh0 * G:(oh0 + 4) * G].rearrange(
                        "p s (oh g) -> p oh s g", g=G)
                    nc.vector.tensor_copy(out=dst, in_=src)

            # Conv accumulation is split per c-chunk: each (cc, s, hi)
            # PSUM group is just 9 matmuls, with the cc=0 partial parked
            # in SBUF (acc) and combined during the cc=1 evacuation. Short
            # group lifetimes keep PSUM-bank pressure low so conv matmuls
            # can run inside the DMA-bound weight-streaming phase.
            acc = {
                (s, hi): accp.tile([P, HH, W], F32, name=f"acc_{s}_{hi}",
                                   tag=f"acc_{s}_{hi}")
                for s in range(S) for hi in range(2)
            }
            _tag = [0]

            def conv_psum():
                t = cvps.tile([P, HH, W], F32, name=f"cps_{_tag[0]}",
                              tag=f"cps_{_tag[0] % 6}")
                _tag[0] += 1
                return t

            def emit_conv_mm(cc, k, s, hi, pst):
                ky, kx = k // KS, k % KS
                h0 = hi * HH
                rhs = xpad[s][cc][:, h0 + ky:h0 + ky + HH, kx:kx + W]
                nc.tensor.matmul(
                    pst, wg[:, cc, k, s, :], rhs,
                    start=(k == 0), stop=(k == KS * KS - 1),
                    skip_group_check=True,
                )

            def emit_group_evac(cc, s, hi, pst):
                if cc == 0:
                    # park cc=0 partial (+bias) in SBUF, on the otherwise
                    # idle scalar engine
                    nc.scalar.activation(
                        acc[(s, hi)], pst,
                        mybir.ActivationFunctionType.Identity,
                        bias=bias_sb[:, 0:1],
                    )
                else:
                    ot = outp.tile([P, HH, W], F32, name=f"ot_{s}_{hi}",
                                   tag="ot")
                    nc.vector.tensor_tensor(
                        ot, pst, acc[(s, hi)], mybir.AluOpType.add,
                    )
                    nc.sync.dma_start(
                        out=out_d[s, :, hi * HH:hi * HH + HH, :], in_=ot,
                    )

            NPROG = 3  # samples whose groups accumulate progressively
            # Phase-1 critical DMAs only: prog x tiles + ALL cc=0 weight
            # blocks, so the PE runs wgen at DMA-arrival cadence. All other
            # loads stream later, under the PE-heavy burst phases.
            for s in range(NPROG):
                emit_xload(s, 0)
            wtbs = {(0, k): emit_wload(0, k) for k in range(KS * KS)}
            for cc in range(CC):
                prog = {
                    (s, hi): conv_psum()
                    for s in range(NPROG) for hi in range(2)
                }
                for k in range(KS * KS):
                    emit_wgen(cc, k, wtbs[(cc, k)])
                    if k >= 2:  # two blocks behind: evac copies of block
                        for s in range(NPROG):  # k-2 finish under k-1, k
                            for hi in range(2):
                                emit_conv_mm(cc, k - 2, s, hi,
                                             prog[(s, hi)])
                for kt in (KS * KS - 2, KS * KS - 1):
                    for s in range(NPROG):
                        for hi in range(2):
                            emit_conv_mm(cc, kt, s, hi, prog[(s, hi)])
                for s in range(NPROG):
                    for hi in range(2):
                        emit_group_evac(cc, s, hi, prog[(s, hi)])
                if cc == 0:
                    # everything the rest of the kernel needs, emitted now
                    # so it streams while PE crunches the cc=0 bursts
                    for s in range(NPROG, S):
                        emit_xload(s, 0)
                    for s in range(NPROG):
                        emit_xload(s, 1)
                    for k in range(KS * KS):
                        wtbs[(1, k)] = emit_wload(1, k)
                    for s in range(NPROG, S):
                        emit_xload(s, 1)
                # burst groups for the remaining samples
                for s in range(NPROG, S):
                    for hi in range(2):
                        pst = conv_psum()
                        for k in range(KS * KS):
                            emit_conv_mm(cc, k, s, hi, pst)
                        emit_group_evac(cc, s, hi, pst)

    nc.compile()
    return nc


def _get_nc():
    global _NC
    if _NC is None:
        _NC = _build_nc()
    return _NC


def _prep_core_inputs(inputs, inputs_se, weight, bias, bg, oh):
    # weight rows: r = o*(C*9) + c*9 + (ky*3+kx)  -> [O, C, 3, 3, NUM]
    wr = weight.reshape(O, C, KS, KS, NUM)
    wo = wr[oh * OC:(oh + 1) * OC]            # [128, 256, 3, 3, 8]
    p_arr = np.arange(NP)
    k_arr = p_arr // OC                       # k index per (m,g) pair
    o_arr = p_arr % OC
    t = wo[o_arr, :, k_arr // KS, k_arr % KS, :]     # [1152, 256, 8]
    wp = (
        t.reshape(NM, G, CC, P, NUM)
        .transpose(2, 0, 4, 1, 3)             # cc, m, n, g, c
        .reshape(CC * NM, P, P)
    )
    wp = np.ascontiguousarray(wp.astype(np.float16))

    se_core = inputs_se[bg * S:(bg + 1) * S]  # [8, 8] (s, n)
    sebd = np.zeros((NUM, G, S, G), dtype=np.float32)
    for g in range(G):
        sebd[:, g, :, g] = se_core.T
    sebd = sebd.reshape(P, P).astype(np.float16)

    x_core = np.pad(
        inputs[bg * S:(bg + 1) * S], ((0, 0), (0, 0), (1, 1), (1, 1))
    )
    return {
        "x": np.ascontiguousarray(x_core.astype(np.float16)),
        "wp": wp,
        "sebd": sebd,
        "bias": np.ascontiguousarray(
            bias[oh * OC:(oh + 1) * OC].reshape(OC, 1), dtype=np.float32
        ),
    }


def kernel(inputs, inputs_se, weight, bias):
    inputs = np.asarray(inputs, dtype=np.float32)
    inputs_se = np.asarray(inputs_se, dtype=np.float32)
    weight = np.asarray(weight, dtype=np.float32)
    bias = np.asarray(bias, dtype=np.float32)

    nc = _get_nc()
    in_maps = []
    for core in range(NCORES):
        bg, oh = core // OHALF, core % OHALF
        in_maps.append(_prep_core_inputs(inputs, inputs_se, weight, bias, bg, oh))

    res = run_bass_kernel_spmd(nc, in_maps, list(range(NCORES))).results

    out = np.empty((B, O, H, W), dtype=np.float32)
    for core in range(NCORES):
        bg, oh = core // OHALF, core % OHALF
        out[bg * S:(bg + 1) * S, oh * OC:(oh + 1) * OC] = res[core]["out"]
    return out


# revision 42
# speedup vs baseline: 1.2416x; 1.0124x over previous
"""Trainium2 Bass kernel for nn_DiverseRegDCConv2d.

Per-sample dynamic 3x3 conv: filters are generated per sample from an
8-column weight bank (wgen[b] = se[b] @ bank.T), then applied as a
standard 256->256 conv on 28x28 with padding 1.

Sharding (8 cores): 4 batch-groups x 2 out-channel halves. Each core
handles 8 samples x 128 out channels; the weight bank half it needs is
replicated across the 4 batch-groups. No cross-device communication.

On-device filter generation trick: the bank half is pre-arranged on the
host into 128x128 stationary tiles whose partition axis is (n, g) with
n = bank column (8) and g = 16 different (k, o)-blocks; the streaming
operand is a block-diagonal arrangement of inputs_se (built on host,
64 KB). One matmul then produces filters for 16 (k,o)-pairs x 8 samples
with the conv's contraction axis (input channel) on PSUM partitions --
exactly the lhsT layout the conv matmuls need, so no on-device
transpose is ever required.

Precision: filter generation and conv both run in fp16 operands
(weights ~N(0, 0.02^2) and x ~N(0,1), well inside fp16 range) with
fp32 PSUM accumulation throughout. End-to-end relative error vs the
fp32 reference is ~7e-4.
"""

import sys

for _p in ("/opt/trn_rl_repo", "/root/.axon_site/_ro/trn_rl_repo"):
    if _p not in sys.path:
        sys.path.append(_p)

import numpy as np

import concourse.bass as bass
import concourse.mybir as mybir
from concourse import bacc
from concourse.bass_utils import run_bass_kernel_spmd
from concourse.tile import TileContext

B, C, O, KS, H, W, NUM = 32, 256, 256, 3, 28, 28, 8
P = 128
NCORES = 8
BG, OHALF = 4, 2          # batch-groups x out-channel halves
S = B // BG               # samples per core = 8
OC = O // OHALF           # out channels per core = 128
CC = C // P               # input-channel chunks = 2
G = 16                    # (k,o)-blocks per wgen matmul (with NUM=8 fills K=128)
NP = KS * KS * OC         # (k, o_local) pairs per c-chunk = 1152
NM = NP // G              # wgen matmuls per c-chunk = 72
F32 = mybir.dt.float32
F32R = mybir.dt.float32r
F16 = mybir.dt.float16

_NC = None


def _build_nc():
    nc = bacc.Bacc()
    x_d = nc.declare_dram_parameter("x", [S, C, H + 2, W + 2], F16, isOutput=False)
    wp_d = nc.declare_dram_parameter("wp", [CC * NM, P, P], F16, isOutput=False)
    se_d = nc.declare_dram_parameter("sebd", [P, P], F16, isOutput=False)
    b_d = nc.declare_dram_parameter("bias", [P, 1], F32, isOutput=False)
    out_d = nc.declare_dram_parameter("out", [S, OC, H, W], F32, isOutput=True)

    with TileContext(nc) as tc:
        with (
            tc.tile_pool(name="constp", bufs=1) as constp,
            tc.tile_pool(name="wstream", bufs=18) as wstream,
            tc.tile_pool(name="slabp", bufs=1) as slabp,
            tc.tile_pool(name="xpool", bufs=1) as xpool,
            tc.tile_pool(name="outp", bufs=8) as outp,
            tc.tile_pool(name="accp", bufs=1) as accp,
            tc.tile_pool(name="wgps", bufs=2, space="PSUM") as wgps,
            tc.tile_pool(name="cvps", bufs=1, space="PSUM") as cvps,
        ):
            se_sb = constp.tile([P, P], F16)
            nc.sync.dma_start(out=se_sb, in_=se_d[:, :])
            bias_sb = constp.tile([P, 1], F32)
            nc.sync.dma_start(out=bias_sb, in_=b_d[:, :])

            # wgen slab: [c_part, cc, k, s, o] -- conv lhsT slices are
            # wg[:, cc, k, s, :], a contiguous [128, 128] tile.
            wg = slabp.tile([P, CC, KS * KS, S, P], F16)

            # padded inputs: per (sample, c-chunk) a [128, 30, 30] tile.
            # Loads are interleaved into the wgen block loop so the weight
            # stream is not starved at kernel start.
            xpad = [[None] * CC for _ in range(S)]
            for s in range(S):
                for cc in range(CC):
                    xpad[s][cc] = xpool.tile(
                        [P, H + 2, W + 2], F16, name=f"xpad_{s}_{cc}",
                        tag=f"xpad_{s}_{cc}",
                    )

            xdone = set()

            def emit_xload(s, cc):
                if (s, cc) in xdone:
                    return
                xdone.add((s, cc))
                nc.sync.dma_start(
                    out=xpad[s][cc], in_=x_d[s, cc * P:(cc + 1) * P, :, :],
                )

            HH = H // 2  # 14 output rows per matmul -> N = 392

            def emit_wload(cc, k):
                # one DMA loads the block's 8 stationary tiles (fp16)
                t0 = cc * NM + k * 8
                wtb = wstream.tile([P, 8, P], F16, name=f"wtb_{cc}_{k}", tag="wtb")
                nc.sync.dma_start(
                    out=wtb,
                    in_=wp_d[t0:t0 + 8, :, :].rearrange("t p c -> p t c"),
                )
                return wtb

            def emit_wgen(cc, k, wtb):
                # produce wg[:, cc, k, :, :] (8 o_hi blocks = 2 psum groups)
                for j in range(2):
                    m0 = k * 8 + j * 4
                    ps = wgps.tile([P, 4 * P], F32)
                    for i in range(4):
                        nc.tensor.matmul(
                            ps[:, i * P:(i + 1) * P], wtb[:, j * 4 + i, :],
                            se_sb, start=True, stop=True,
                        )
                    # psum free layout: (o_hi, s, g); slab wants (s, o_hi, g)
                    oh0 = m0 % 8
                    src = ps.rearrange("p (oh s g) -> p oh s g", oh=4, s=S, g=G)
                    dst = wg[:, cc, k, :, oh0 * G:(oh0 + 4) * G].rearrange(
                        "p s (oh g) -> p oh s g", g=G)
                    nc.vector.tensor_copy(out=dst, in_=src)

            # Conv accumulation is split per c-chunk: each (cc, s, hi)
            # PSUM group is just 9 matmuls, with the cc=0 partial parked
            # in SBUF (acc) and combined during the cc=1 evacuation. Short
            # group lifetimes keep PSUM-bank pressure low so conv matmuls
            # can run inside the DMA-bound weight-streaming phase.
            acc = {
                (s, hi): accp.tile([P, HH, W], F32, name=f"acc_{s}_{hi}",
                                   tag=f"acc_{s}_{hi}")
                for s in range(S) for hi in range(2)
            }
            _tag = [0]

            def conv_psum():
                t = cvps.tile([P, HH, W], F32, name=f"cps_{_tag[0]}",
                              tag=f"cps_{_tag[0] % 6}")
                _tag[0] += 1
                return t

            def emit_conv_mm(cc, k, s, hi, pst):
                ky, kx = k // KS, k % KS
                h0 = hi * HH
                rhs = xpad[s][cc][:, h0 + ky:h0 + ky + HH, kx:kx + W]
                nc.tensor.matmul(
                    pst, wg[:, cc, k, s, :], rhs,
                    start=(k == 0), stop=(k == KS * KS - 1),
                    skip_group_check=True,
                )

            def emit_group_evac(cc, s, hi, pst):
                if cc == 0:
                    # park cc=0 partial (+bias) in SBUF, on the otherwise
                    # idle scalar engine
                    nc.scalar.activation(
                        acc[(s, hi)], pst,
                        mybir.ActivationFunctionType.Identity,
                        bias=bias_sb[:, 0:1],
                    )
                else:
                    ot = outp.tile([P, HH, W], F32, name=f"ot_{s}_{hi}",
                                   tag="ot")
                    nc.vector.tensor_tensor(
                        ot, pst, acc[(s, hi)], mybir.AluOpType.add,
                    )
                    nc.sync.dma_start(
                        out=out_d[s, :, hi * HH:hi * HH + HH, :], in_=ot,
                    )

            NPROG = 3  # samples whose groups accumulate progressively
            # Phase-1 critical DMAs only: prog x tiles + ALL cc=0 weight
            # blocks, so the PE runs wgen at DMA-arrival cadence. All other
            # loads stream later, under the PE-heavy burst phases.
            for s in range(NPROG):
                emit_xload(s, 0)
            wtbs = {(0, k): emit_wload(0, k) for k in range(KS * KS)}
            for cc in range(CC):
                prog = {
                    (s, hi): conv_psum()
                    for s in range(NPROG) for hi in range(2)
                }
                for k in range(KS * KS):
                    if cc == 0:  # cc=1 wgen runs inside the cc=0 bursts
                        emit_wgen(cc, k, wtbs[(cc, k)])
                    if k >= 2:  # two blocks behind: evac copies of block
                        for s in range(NPROG):  # k-2 finish under k-1, k
                            for hi in range(2):
                                emit_conv_mm(cc, k - 2, s, hi,
                                             prog[(s, hi)])
                for kt in (KS * KS - 2, KS * KS - 1):
                    for s in range(NPROG):
                        for hi in range(2):
                            emit_conv_mm(cc, kt, s, hi, prog[(s, hi)])
                for s in range(NPROG):
                    for hi in range(2):
                        emit_group_evac(cc, s, hi, prog[(s, hi)])
                if cc == 0:
                    # everything the rest of the kernel needs, emitted now
                    # so it streams while PE crunches the cc=0 bursts
                    for s in range(NPROG, S):
                        emit_xload(s, 0)
                    for s in range(NPROG):
                        emit_xload(s, 1)
                    for k in range(KS * KS):
                        wtbs[(1, k)] = emit_wload(1, k)
                    for s in range(NPROG, S):
                        emit_xload(s, 1)
                # burst groups for the remaining samples; during the cc=0
                # bursts the cc=1 filter generation is interleaved (PE has
                # burst matmuls to hide the wgen->copy chain, DVE is idle)
                bidx = 0
                for s in range(NPROG, S):
                    for hi in range(2):
                        if cc == 0 and bidx < KS * KS:
                            emit_wgen(1, bidx, wtbs[(1, bidx)])
                        bidx += 1
                        pst = conv_psum()
                        for k in range(KS * KS):
                            emit_conv_mm(cc, k, s, hi, pst)
                        emit_group_evac(cc, s, hi, pst)

    nc.compile()
    return nc


def _get_nc():
    global _NC
    if _NC is None:
        _NC = _build_nc()
    return _NC


def _prep_core_inputs(inputs, inputs_se, weight, bias, bg, oh):
    # weight rows: r = o*(C*9) + c*9 + (ky*3+kx)  -> [O, C, 3, 3, NUM]
    wr = weight.reshape(O, C, KS, KS, NUM)
    wo = wr[oh * OC:(oh + 1) * OC]            # [128, 256, 3, 3, 8]
    p_arr = np.arange(NP)
    k_arr = p_arr // OC                       # k index per (m,g) pair
    o_arr = p_arr % OC
    t = wo[o_arr, :, k_arr // KS, k_arr % KS, :]     # [1152, 256, 8]
    wp = (
        t.reshape(NM, G, CC, P, NUM)
        .transpose(2, 0, 4, 1, 3)             # cc, m, n, g, c
        .reshape(CC * NM, P, P)
    )
    wp = np.ascontiguousarray(wp.astype(np.float16))

    se_core = inputs_se[bg * S:(bg + 1) * S]  # [8, 8] (s, n)
    sebd = np.zeros((NUM, G, S, G), dtype=np.float32)
    for g in range(G):
        sebd[:, g, :, g] = se_core.T
    sebd = sebd.reshape(P, P).astype(np.float16)

    x_core = np.pad(
        inputs[bg * S:(bg + 1) * S], ((0, 0), (0, 0), (1, 1), (1, 1))
    )
    return {
        "x": np.ascontiguousarray(x_core.astype(np.float16)),
        "wp": wp,
        "sebd": sebd,
        "bias": np.ascontiguousarray(
            bias[oh * OC:(oh + 1) * OC].reshape(OC, 1), dtype=np.float32
        ),
    }


def kernel(inputs, inputs_se, weight, bias):
    inputs = np.asarray(inputs, dtype=np.float32)
    inputs_se = np.asarray(inputs_se, dtype=np.float32)
    weight = np.asarray(weight, dtype=np.float32)
    bias = np.asarray(bias, dtype=np.float32)

    nc = _get_nc()
    in_maps = []
    for core in range(NCORES):
        bg, oh = core // OHALF, core % OHALF
        in_maps.append(_prep_core_inputs(inputs, inputs_se, weight, bias, bg, oh))

    res = run_bass_kernel_spmd(nc, in_maps, list(range(NCORES))).results

    out = np.empty((B, O, H, W), dtype=np.float32)
    for core in range(NCORES):
        bg, oh = core // OHALF, core % OHALF
        out[bg * S:(bg + 1) * S, oh * OC:(oh + 1) * OC] = res[core]["out"]
    return out
